# revision 10
# baseline (speedup 1.0000x reference)
"""AttentionDAF Trainium2 kernel — data-parallel over batch across 8 NeuronCores.

Reference (per batch element, c=inputs (512,768), q=states (512,768)):
    s[i,j] = (c*wcq)@q^T + s_c[i] + s_q[j] + mask
    a = softmax_j(s);  c2q = a @ q
    b = softmax_i(max_j s);  q2c = b @ c (broadcast over rows)
    x = [c, c2q, c*c2q, c*q2c];  y = relu(x @ wa^T + wa_b) + c;  out = LN(y)

Math/HW structure used here:
  - Softmax row constants (s_c, all biases) drop out of `a`; only
    s0 = (c*wcq)@q^T + s_q matters. Logits computed TRANSPOSED (s0T[j,i]) so
    s_q is a per-partition bias of the exp activation, which writes
    E^T = exp(s0T + s_q) directly as fp8.
  - Row normalization (1/rowsum) is deferred: unnormalized c2q_un/xc_un flow
    through the fp8 matmuls; the per-row scalar 1/(2*rs[i]) is applied at the
    PSUM combine (scalar_tensor_tensor, per-partition scalar).
  - max_j s for the b-path = ln(max_j E^T): TT-max over j-chunks + identity
    transposes + free-axis max. b stays unnormalized; 1/sum(b) folds into the
    q2c column transpose.
  - fp8e4 (e4m3) DoubleRow matmuls (2 K-chunks/instr, 2x PE throughput) for:
    s0T, rowsum, s_q/s_c rows, c2qT, and the c2q/xc components of the final
    matmul. The precision-critical c-component (rhs = wa1 + q2c.*wa4) stays
    bf16. fp8 operands are pre-scaled into e4m3's normal range (64x on
    weight-like vectors, 1/32 on c2q_un); scales cancel at the combine.
Per-core work: 2 batch elements, no collectives.
"""
import sys
from contextlib import ExitStack

if "/opt/trn_rl_repo" not in sys.path:
    sys.path.insert(0, "/opt/trn_rl_repo")

import numpy as np
import ml_dtypes

from concourse import bacc
import concourse.bacc as bacc_mod
import concourse.hw_specs as hw_specs
import concourse.bass as bass
import concourse.tile as tile
import concourse.mybir as mybir
from concourse.bass_utils import run_bass_kernel_spmd
from concourse.masks import make_identity

F32 = mybir.dt.float32
BF16 = mybir.dt.bfloat16
F8 = mybir.dt.float8e4
AF = mybir.ActivationFunctionType
X = mybir.AxisListType.X
MULT = mybir.AluOpType.mult
ADD = mybir.AluOpType.add
SUB = mybir.AluOpType.subtract
MAXOP = mybir.AluOpType.max
BYPASS = mybir.AluOpType.bypass
DR = mybir.MatmulPerfMode.DoubleRow

B, CL, QL, H = 16, 512, 512, 768
N_CORES = 8
BPC = B // N_CORES      # batch elements per core
PC = CL // 128          # i-chunks (c rows)
QC = QL // 128          # j-chunks (q rows)
HC = H // 128           # h-chunks
LN_EPS = 1e-5
BF = ml_dtypes.bfloat16
E4M3 = ml_dtypes.float8_e4m3

# All activation funcs we use (Exp, Ln, Copy, Square) live in the
# "natural_log_exp_and_others" table set. Blank out every other set so bass's
# table-load inserter emits exactly one load (2.7us per switch otherwise).
_ORIG_GAT = hw_specs.get_activation_tables


def _single_set_tables(arch):
    t = _ORIG_GAT(arch)
    return {
        name: (funcs if name == "natural_log_exp_and_others" else set())
        for name, funcs in t.items()
    }


bacc_mod.get_activation_tables = _single_set_tables


def build_kernel(use_mask: bool, trivial_ln: bool, reps: int = 1):
    nc = bacc.Bacc("TRN2", target_bir_lowering=False, debug=False)

    # ---- DRAM I/O (per-core shard shapes) ----
    d_cn = nc.dram_tensor("cn", [BPC, CL, H], BF16, kind="ExternalInput")
    d_cT = nc.dram_tensor("cT", [BPC, H, CL], BF16, kind="ExternalInput")
    d_cT8 = nc.dram_tensor("cT8", [BPC, H, CL], F8, kind="ExternalInput")
    d_qT8 = nc.dram_tensor("qT8", [BPC, H, QL], F8, kind="ExternalInput")
    d_qn8 = nc.dram_tensor("qn8", [BPC, QL, H], F8, kind="ExternalInput")
    d_wcq = nc.dram_tensor("wcq64", [128, HC], F32, kind="ExternalInput")
    d_wc8 = nc.dram_tensor("wc8", [128, HC, 1], F8, kind="ExternalInput")
    d_wq8 = nc.dram_tensor("wq8", [128, HC, 1], F8, kind="ExternalInput")
    d_waB = nc.dram_tensor("waB", [128, 2 * HC, H], BF16, kind="ExternalInput")
    d_waA = nc.dram_tensor("waA", [128, 2 * HC, H], F8, kind="ExternalInput")
    d_wab = nc.dram_tensor("wab", [1, H], BF16, kind="ExternalInput")
    if use_mask:
        d_mkT = nc.dram_tensor("mkT", [BPC, QL, CL], F32, kind="ExternalInput")
    if not trivial_ln:
        d_lng = nc.dram_tensor("lng", [H], F32, kind="ExternalInput")
        d_lnb = nc.dram_tensor("lnb", [H], F32, kind="ExternalInput")
    d_out = nc.dram_tensor("out", [BPC, CL, H], BF16, kind="ExternalOutput")

    with tile.TileContext(nc) as tc, ExitStack() as ctx:
        consts = ctx.enter_context(tc.tile_pool(name="consts", bufs=1))
        p_in = ctx.enter_context(tc.tile_pool(name="pin", bufs=2))
        p_work = ctx.enter_context(tc.tile_pool(name="work", bufs=2))
        p_small = ctx.enter_context(tc.tile_pool(name="small", bufs=2))
        p_y = ctx.enter_context(tc.tile_pool(name="ypool", bufs=2))
        # PSUM: 8 banks of [128 x 2KB]:
        #   mm  [128,512]f32 x2 bufs = 2 banks (s0T / c2qT / misc)
        #   row [<=128,<=512] x1     = 1 bank  (rows, transposes, cols)
        #   B1 [128,512] x2 = 2, B2 [128,256] x2 = 1   (bf16 group)
        #   A1 [128,512] x1 = 1, A2 [128,256] x1 = 1   (fp8 group)
        ps_mm = ctx.enter_context(tc.tile_pool(name="ps_mm", bufs=2, space="PSUM"))
        ps_row = ctx.enter_context(tc.tile_pool(name="ps_row", bufs=1, space="PSUM"))
        ps_b1 = ctx.enter_context(tc.tile_pool(name="ps_b1", bufs=2, space="PSUM"))
        ps_b2 = ctx.enter_context(tc.tile_pool(name="ps_b2", bufs=1, space="PSUM"))
        ps_a1 = ctx.enter_context(tc.tile_pool(name="ps_a1", bufs=1, space="PSUM"))
        ps_a2 = ctx.enter_context(tc.tile_pool(name="ps_a2", bufs=1, space="PSUM"))

        # ---- constants (once per core) ----
        waB = consts.tile([128, 2 * HC, H], BF16)      # [wa1 | wa4]
        nc.scalar.dma_start(waB[:], d_waB.ap()[:])
        waA = consts.tile([128, 2 * HC, H], F8)        # 64*[wa2 | wa3]
        nc.scalar.dma_start(waA[:], d_waA.ap()[:])
        wcq_c = consts.tile([128, HC], F32)            # 64*wcq columns
        nc.sync.dma_start(wcq_c[:], d_wcq.ap()[:])
        wc8 = consts.tile([128, HC, 1], F8)            # 64*wc columns
        nc.sync.dma_start(wc8[:], d_wc8.ap()[:])
        wq8 = consts.tile([128, HC, 1], F8)            # 64*wq columns
        nc.sync.dma_start(wq8[:], d_wq8.ap()[:])
        wab_pad = consts.tile([128, H], BF16)
        nc.vector.memset(wab_pad[:], 0.0)
        nc.sync.dma_start(wab_pad[0:1, :], d_wab.ap()[:])
        id_bf = consts.tile([128, 128], BF16)
        make_identity(nc, id_bf[:])
        ones_t = consts.tile([128, 128], BF16)         # row-0 ones (bias bcast)
        nc.vector.memset(ones_t[:], 0.0)
        nc.vector.memset(ones_t[0:1, :], 1.0)
        ones8 = consts.tile([128, 2, 1], F8)           # all-ones (rowsum lhsT)
        nc.vector.memset(ones8[:], 1.0)
        ones_col = consts.tile([128, 1], BF16)         # all-ones column
        nc.vector.memset(ones_col[:], 1.0)
        c64i = consts.tile([1, 1], F32)                # 1/64  (unscale rows)
        nc.vector.memset(c64i[:], 1.0 / 64.0)
        c05 = consts.tile([1, 1], F32)                 # 0.5   (1/(fp8 scale S))
        nc.vector.memset(c05[:], 0.5)
        eps_t = consts.tile([128, 1], F32)
        nc.vector.memset(eps_t[:], LN_EPS)
        if not trivial_ln:
            g_bc = consts.tile([128, H], F32)
            nc.sync.dma_start(
                g_bc[:], bass.AP(tensor=d_lng, offset=0, ap=[[0, 128], [1, H]])
            )
            b_bc = consts.tile([128, H], F32)
            nc.sync.dma_start(
                b_bc[:], bass.AP(tensor=d_lnb, offset=0, ap=[[0, 128], [1, H]])
            )

        rep_ctx = tc.For_i(0, reps, 1) if reps > 1 else None
        if rep_ctx is not None:
            rep_ctx.__enter__()
        for b in range(BPC):
            # ---- per-batch loads ----
            qT8 = p_in.tile([128, HC, QL], F8, tag="qT8")
            nc.sync.dma_start(qT8[:], d_qT8.ap()[b].rearrange("(o p) j -> p o j", p=128))
            cT8 = p_in.tile([128, HC, CL], F8, tag="cT8")
            nc.sync.dma_start(cT8[:], d_cT8.ap()[b].rearrange("(o p) i -> p o i", p=128))
            cTb = p_in.tile([128, HC, CL], BF16, tag="cTb")
            nc.scalar.dma_start(cTb[:], d_cT.ap()[b].rearrange("(o p) i -> p o i", p=128))
            qn8 = p_in.tile([128, QC, H], F8, tag="qn8")
            nc.scalar.dma_start(qn8[:], d_qn8.ap()[b].rearrange("(o p) h -> p o h", p=128))
            cn = p_in.tile([128, PC, H], BF16, tag="cn")
            nc.sync.dma_start(cn[:], d_cn.ap()[b].rearrange("(o p) h -> p o h", p=128))
            if use_mask:
                mkT = p_in.tile([128, QC, CL], F32, tag="mkT")
                nc.sync.dma_start(
                    mkT[:], d_mkT.ap()[b].rearrange("(o p) i -> p o i", p=128)
                )

            # ---- csT8 = 64*wcq (.) c^T   (fp8, per-partition scale) ----
            csT8 = p_work.tile([128, HC, CL], F8, tag="csT8")
            for hc in range(HC):
                nc.scalar.activation(
                    out=csT8[:, hc], in_=cTb[:, hc], func=AF.Copy,
                    scale=wcq_c[:, hc : hc + 1],
                )

            # ---- s_q columns: 64*s_q row (fp8 DR) -> SBUF -> cols /64 ----
            sq_ps = ps_row.tile([1, QL], F32, tag="row")
            for k in range(HC):
                nc.tensor.matmul(
                    sq_ps[:], lhsT=wq8[:, k, :], rhs=qT8[:, k],
                    start=(k == 0), stop=(k == HC - 1),
                )
            sq_sb = p_small.tile([1, QL], F32, tag="sq_sb")
            nc.scalar.copy(sq_sb[:], sq_ps[:])
            sqc_ps = ps_row.tile([128, QC], F32, tag="row")
            for jc in range(QC):
                nc.tensor.matmul(
                    sqc_ps[:, jc : jc + 1],
                    lhsT=sq_sb[0:1, jc * 128 : (jc + 1) * 128],
                    rhs=c64i[:], start=True, stop=True,
                )
            sqc = p_small.tile([128, QC], F32, tag="sqc")
            nc.scalar.copy(sqc[:], sqc_ps[:])

            # ---- s0T per j-chunk (fp8 DR) -> E^T = exp(s0T/64 + s_q) fp8 ----
            ET8 = p_work.tile([128, QC, CL], F8, tag="ET8")
            for jc in range(QC):
                s0 = ps_mm.tile([128, CL], F32, tag="mm")
                for k in range(0, HC, 2):
                    nc.tensor.matmul(
                        s0[:],
                        lhsT=qT8[:, k : k + 2, jc * 128 : (jc + 1) * 128],
                        rhs=csT8[:, k : k + 2],
                        start=(k == 0), stop=(k == HC - 2), perf_mode=DR,
                    )
                if use_mask:
                    nc.vector.tensor_add(s0[:], s0[:], mkT[:, jc])
                nc.scalar.activation(
                    out=ET8[:, jc], in_=s0[:], func=AF.Exp,
                    bias=sqc[:, jc : jc + 1], scale=1.0 / 64.0,
                )

            # ---- rowsum rs[i] (fp8 DR ones) -> rr = 1/rs -> cols * 0.5 ----
            rs_ps = ps_row.tile([1, CL], F32, tag="row")
            for k in range(QC):
                nc.tensor.matmul(
                    rs_ps[:], lhsT=ones8[:, 0, :], rhs=ET8[:, k],
                    start=(k == 0), stop=(k == QC - 1),
                )
            rr_row = p_small.tile([1, CL], F32, tag="rr_row")
            nc.vector.reciprocal(rr_row[:], rs_ps[:])
            rrc_ps = ps_row.tile([128, PC], F32, tag="row")
            for ic in range(PC):
                nc.tensor.matmul(
                    rrc_ps[:, ic : ic + 1],
                    lhsT=rr_row[0:1, ic * 128 : (ic + 1) * 128],
                    rhs=c05[:], start=True, stop=True,
                )
            rrS = p_small.tile([128, PC], F32, tag="rrS")
            nc.scalar.copy(rrS[:], rrc_ps[:])

            # ---- column max over j: ln(max_j E^T) path for b ----
            mx1 = p_small.tile([128, CL], BF16, tag="mx1")
            mx2 = p_small.tile([128, CL], BF16, tag="mx2")
            nc.vector.tensor_tensor(mx1[:], ET8[:, 0], ET8[:, 1], op=MAXOP)
            nc.vector.tensor_tensor(mx2[:], ET8[:, 2], ET8[:, 3], op=MAXOP)
            nc.vector.tensor_tensor(mx1[:], mx1[:], mx2[:], op=MAXOP)
            mxE = p_small.tile([128, PC], F32, tag="mxE")
            for ic in range(PC):
                mt_ps = ps_row.tile([128, 128], F32, tag="row")
                nc.tensor.matmul(
                    mt_ps[:], lhsT=mx1[:, ic * 128 : (ic + 1) * 128],
                    rhs=id_bf[:], start=True, stop=True,
                )
                nc.vector.tensor_reduce(
                    out=mxE[:, ic : ic + 1], in_=mt_ps[:], axis=X, op=MAXOP
                )

            # ---- b (unnormalized): eb = exp(s_c) * maxE; 1/sum via rb ----
            sc_ps = ps_row.tile([1, CL], F32, tag="row")
            for k in range(HC):
                nc.tensor.matmul(
                    sc_ps[:], lhsT=wc8[:, k, :], rhs=cT8[:, k],
                    start=(k == 0), stop=(k == HC - 1),
                )
            sc_sb = p_small.tile([1, CL], F32, tag="sc_sb")
            nc.scalar.copy(sc_sb[:], sc_ps[:])
            scc_ps = ps_row.tile([128, PC], F32, tag="row")
            for ic in range(PC):
                nc.tensor.matmul(
                    scc_ps[:, ic : ic + 1],
                    lhsT=sc_sb[0:1, ic * 128 : (ic + 1) * 128],
                    rhs=c64i[:], start=True, stop=True,
                )
            esc = p_small.tile([128, PC], F32, tag="esc")
            nc.scalar.activation(out=esc[:], in_=scc_ps[:], func=AF.Exp)
            eb = p_small.tile([128, PC], BF16, tag="eb")
            nc.vector.tensor_tensor(eb[:], esc[:], mxE[:], op=MULT)
            ebs_ps = ps_row.tile([1, PC], F32, tag="row")
            nc.tensor.matmul(ebs_ps[:], lhsT=ones_col[:], rhs=eb[:], start=True, stop=True)
            Sb = p_small.tile([1, 1], F32, tag="Sb")
            nc.vector.tensor_reduce(out=Sb[:], in_=ebs_ps[:], axis=X, op=ADD)
            rb = p_small.tile([1, 1], F32, tag="rb")
            nc.vector.reciprocal(rb[:], Sb[:])

            # ---- c2qT (unnorm, fp8 DR) -> c2qT8 = /32; xc8 = cT8 * c2qT8 ----
            c2qT8 = p_work.tile([128, HC, CL], F8, tag="c2qT8")
            xc8 = p_work.tile([128, HC, CL], F8, tag="xc8")
            for hc in range(HC):
                cq_ps = ps_mm.tile([128, CL], F32, tag="mm")
                for k in range(0, QC, 2):
                    nc.tensor.matmul(
                        cq_ps[:],
                        lhsT=qn8[:, k : k + 2, hc * 128 : (hc + 1) * 128],
                        rhs=ET8[:, k : k + 2],
                        start=(k == 0), stop=(k == QC - 2), perf_mode=DR,
                    )
                nc.scalar.activation(
                    out=c2qT8[:, hc], in_=cq_ps[:], func=AF.Copy, scale=1.0 / 32.0
                )
                nc.gpsimd.tensor_tensor(xc8[:, hc], cT8[:, hc], c2qT8[:, hc], op=MULT)

            # ---- q2c row = eb @ c (bf16); cols scaled by rb -> true q2c ----
            q2c_sb = p_small.tile([1, H], F32, tag="q2c_sb")
            for n0, nw in ((0, 512), (512, 256)):
                qp = ps_row.tile([1, nw], F32, tag="row")
                for ic in range(PC):
                    nc.tensor.matmul(
                        qp[:], lhsT=eb[:, ic : ic + 1],
                        rhs=cn[:, ic, n0 : n0 + nw],
                        start=(ic == 0), stop=(ic == PC - 1),
                    )
                nc.scalar.copy(q2c_sb[0:1, n0 : n0 + nw], qp[:])
            qcc_ps = ps_row.tile([128, HC], F32, tag="row")
            for hc in range(HC):
                nc.tensor.matmul(
                    qcc_ps[:, hc : hc + 1],
                    lhsT=q2c_sb[0:1, hc * 128 : (hc + 1) * 128],
                    rhs=rb[:], start=True, stop=True,
                )
            q2c_c = p_small.tile([128, HC], F32, tag="q2c_c")
            nc.scalar.copy(q2c_c[:], qcc_ps[:])

            # ---- merged = wa1 + q2c (.) wa4  (c-component rhs, bf16) ----
            merged = p_work.tile([128, HC, H], BF16, tag="merged")
            for hc in range(HC):
                nc.vector.scalar_tensor_tensor(
                    out=merged[:, hc], in0=waB[:, HC + hc],
                    scalar=q2c_c[:, hc : hc + 1], in1=waB[:, hc],
                    op0=MULT, op1=ADD,
                )

            # ---- final matmul: B group (bf16 c@merged + bias), A group (fp8) ----
            yt = p_y.tile([128, PC, H], BF16, tag="y")
            ysum = p_small.tile([128, PC], F32, tag="ysum")
            ysq = p_small.tile([128, PC], F32, tag="ysq")
            vt = p_y.tile([128, H], BF16, tag="vt")
            av = p_y.tile([128, H], BF16, tag="av")
            for ic in range(PC):
                pb1 = ps_b1.tile([128, 512], F32, tag="b1")
                pb2 = ps_b2.tile([128, 256], F32, tag="b2")
                pa1 = ps_a1.tile([128, 512], F32, tag="a1")
                pa2 = ps_a2.tile([128, 256], F32, tag="a2")
                pb = [pb1, pb2]
                pa = [pa1, pa2]
                for ni, (n0, nw) in enumerate(((0, 512), (512, 256))):
                    for hc in range(HC):
                        nc.tensor.matmul(
                            pb[ni][:],
                            lhsT=cTb[:, hc, ic * 128 : (ic + 1) * 128],
                            rhs=merged[:, hc, n0 : n0 + nw],
                            start=(hc == 0), stop=False,
                            skip_group_check=True,
                        )
                    nc.tensor.matmul(
                        pb[ni][:], lhsT=ones_t[:], rhs=wab_pad[:, n0 : n0 + nw],
                        start=False, stop=True, skip_group_check=True,
                    )
                    k2 = 0
                    for comp, w0 in ((c2qT8, 0), (xc8, HC)):
                        for k in range(0, HC, 2):
                            nc.tensor.matmul(
                                pa[ni][:],
                                lhsT=comp[:, k : k + 2, ic * 128 : (ic + 1) * 128],
                                rhs=waA[:, w0 + k : w0 + k + 2, n0 : n0 + nw],
                                start=(k2 == 0), stop=(k2 == 2 * HC - 4),
                                perf_mode=DR, skip_group_check=True,
                            )
                            k2 += 2
                    # v = bigA * (rr/2) + bigB  (Pool scales PSUM A -> SBUF;
                    # DVE adds PSUM B — only one PSUM operand allowed per op)
                    nc.scalar.activation(
                        out=av[:, n0 : n0 + nw], in_=pa[ni][:], func=AF.Copy,
                        scale=rrS[:, ic : ic + 1],
                    )
                    nc.vector.scalar_tensor_tensor(
                        out=vt[:, n0 : n0 + nw], in0=pb[ni][:], scalar=0.0,
                        in1=av[:, n0 : n0 + nw], op0=BYPASS, op1=ADD,
                    )
                # y = relu(v) + c ; accumulate sum(y) and sum(y^2)
                nc.vector.scalar_tensor_tensor(
                    out=yt[:, ic], in0=vt[:], scalar=0.0, in1=cn[:, ic],
                    op0=MAXOP, op1=ADD, accum_out=ysum[:, ic : ic + 1],
                )
                nc.vector.scalar_tensor_tensor(
                    out=vt[:], in0=yt[:, ic], scalar=1.0, in1=yt[:, ic],
                    op0=BYPASS, op1=MULT, accum_out=ysq[:, ic : ic + 1],
                )

            # ---- layernorm stats (batched over the 4 i-chunks) ----
            mu = p_small.tile([128, PC], F32, tag="mu")
            nc.scalar.activation(out=mu[:], in_=ysum[:], func=AF.Copy, scale=1.0 / H)
            var = p_small.tile([128, PC], F32, tag="var")
            nc.vector.scalar_tensor_tensor(
                out=var[:], in0=mu[:], scalar=1.0, in1=mu[:], op0=BYPASS, op1=MULT
            )
            nc.vector.scalar_tensor_tensor(
                out=var[:], in0=ysq[:], scalar=1.0 / H, in1=var[:], op0=MULT, op1=SUB
            )
            lnv = p_small.tile([128, PC], F32, tag="lnv")
            nc.scalar.activation(
                out=lnv[:], in_=var[:], func=AF.Ln, bias=eps_t[:], scale=1.0
            )
            rstd = p_small.tile([128, PC], F32, tag="rstd")
            nc.scalar.activation(out=rstd[:], in_=lnv[:], func=AF.Exp, scale=-0.5)
            ot = p_y.tile([128, PC, H], BF16, tag="ot")
            for ic in range(PC):
                nc.vector.tensor_scalar(
                    out=ot[:, ic], in0=yt[:, ic],
                    scalar1=mu[:, ic : ic + 1], scalar2=rstd[:, ic : ic + 1],
                    op0=SUB, op1=MULT,
                )
                if not trivial_ln:
                    nc.vector.tensor_tensor(ot[:, ic], ot[:, ic], g_bc[:], op=MULT)
                    nc.vector.tensor_add(ot[:, ic], ot[:, ic], b_bc[:])
                nc.sync.dma_start(
                    d_out.ap()[b].rearrange("(o p) h -> p o h", p=128)[:, ic],
                    ot[:, ic],
                )

        if rep_ctx is not None:
            rep_ctx.__exit__(None, None, None)

    nc.compile()
    return nc


_KERNEL_CACHE = {}


def get_kernel(use_mask: bool, trivial_ln: bool):
    key = (use_mask, trivial_ln)
    if key not in _KERNEL_CACHE:
        _KERNEL_CACHE[key] = build_kernel(use_mask, trivial_ln)
    return _KERNEL_CACHE[key]


def prep_inputs(inputs):
    """Host-side layout prep: shard over batch, transpose/cast, weight reshape."""
    c = np.ascontiguousarray(np.asarray(inputs["inputs"], dtype=np.float32))
    q = np.ascontiguousarray(np.asarray(inputs["states"], dtype=np.float32))
    mask = np.asarray(inputs["attention_mask"], dtype=np.float32)[:, 0]
    use_mask = bool(np.any(mask))
    ln_g = np.asarray(inputs["ln_g"], dtype=np.float32)
    ln_b = np.asarray(inputs["ln_b"], dtype=np.float32)
    trivial_ln = bool(np.all(ln_g == 1.0) and np.all(ln_b == 0.0))

    cn = c.astype(BF)
    cT = np.ascontiguousarray(c.transpose(0, 2, 1))
    qT = np.ascontiguousarray(q.transpose(0, 2, 1))
    cTb = cT.astype(BF)
    cT8 = cT.astype(E4M3)
    qT8 = qT.astype(E4M3)
    qn8 = q.astype(E4M3)

    def cols(w, scale):
        return np.ascontiguousarray(
            (np.asarray(w, np.float32)[0] * scale).reshape(HC, 128).T
        )

    wcq64 = cols(inputs["wcq_w"], 64.0)
    wc8 = cols(inputs["wc_w"], 64.0).astype(E4M3).reshape(128, HC, 1)
    wq8 = cols(inputs["wq_w"], 64.0).astype(E4M3).reshape(128, HC, 1)
    # waT chunks: [128, 4*HC, H] with chunk f = rows 128f..128(f+1) of wa^T
    waT = np.ascontiguousarray(
        np.asarray(inputs["wa_w"], np.float32).T.reshape(4 * HC, 128, H)
        .transpose(1, 0, 2)
    )
    waB = np.ascontiguousarray(
        np.concatenate([waT[:, :HC], waT[:, 3 * HC :]], axis=1)
    ).astype(BF)                          # [wa1 | wa4]
    waA = np.ascontiguousarray(64.0 * waT[:, HC : 3 * HC]).astype(E4M3)  # 64*[wa2|wa3]
    wab = np.asarray(inputs["wa_b"], np.float32).reshape(1, H).astype(BF)

    in_maps = []
    for k in range(N_CORES):
        sl = slice(k * BPC, (k + 1) * BPC)
        m = {
            "cn": cn[sl],
            "cT": cTb[sl],
            "cT8": cT8[sl],
            "qT8": qT8[sl],
            "qn8": qn8[sl],
            "wcq64": wcq64,
            "wc8": wc8,
            "wq8": wq8,
            "waB": waB,
            "waA": waA,
            "wab": wab,
        }
        if use_mask:
            m["mkT"] = np.ascontiguousarray(
                64.0 * mask[sl].transpose(0, 2, 1)
            )
        if not trivial_ln:
            m["lng"] = ln_g
            m["lnb"] = ln_b
        in_maps.append(m)
    return in_maps, use_mask, trivial_ln


def kernel(**inputs) -> np.ndarray:
    in_maps, use_mask, trivial_ln = prep_inputs(inputs)
    nc = get_kernel(use_mask, trivial_ln)
    res = run_bass_kernel_spmd(nc, in_maps, core_ids=list(range(N_CORES)))
    return np.concatenate(
        [np.asarray(res.results[k]["out"]).astype(np.float32) for k in range(N_CORES)],
        axis=0,
    )


# revision 13
# speedup vs baseline: 1.1314x; 1.1314x over previous
"""AttentionDAF Trainium2 kernel — data-parallel over batch across 8 NeuronCores.

Reference (per batch element, c=inputs (512,768), q=states (512,768)):
    s[i,j] = (c*wcq)@q^T + s_c[i] + s_q[j] + mask
    a = softmax_j(s);  c2q = a @ q
    b = softmax_i(max_j s);  q2c = b @ c (broadcast over rows)
    x = [c, c2q, c*c2q, c*q2c];  y = relu(x @ wa^T + wa_b) + c;  out = LN(y)

Structure:
  - Logits computed TRANSPOSED (s0T[j,i]) so s_q is a per-partition bias of
    the exp activation, which writes E^T = exp(s0T + s_q) directly as fp8.
  - Row normalization (1/rowsum) is deferred: unnormalized c2q_un/xc_un flow
    through the fp8 matmuls; the per-row scalar rr/2 applies at the PSUM
    combine (Activation copy with per-partition scale).
  - max_j s for the b-path = max_j E^T (exp monotonic): TT-max over j-chunks,
    identity-transpose blocks, free-axis max. b stays unnormalized; 1/sum(b)
    folds into the q2c column transpose.
  - fp8e4 DoubleRow matmuls (2 K-chunks/instr, ~2x bf16 PE throughput) for
    s0T, c2qT, row matmuls (M padded to 128), and the c2q/xc components of
    the final matmul. The precision-critical c-component (rhs = wa1 +
    q2c.*wa4, built via one STT per chunk) stays bf16. fp8 operands are
    pre-scaled into e4m3's normal range; scales cancel at the combine.
  - The two batch elements per core are emitted stage-interleaved so the
    in-order engine queues overlap batch 0's dependency stalls with batch 1
    work (and the For_i rep loop pipelines across iterations).
"""
import sys
from contextlib import ExitStack

if "/opt/trn_rl_repo" not in sys.path:
    sys.path.insert(0, "/opt/trn_rl_repo")

import numpy as np
import ml_dtypes

from concourse import bacc
import concourse.bacc as bacc_mod
import concourse.hw_specs as hw_specs
import concourse.bass as bass
import concourse.tile as tile
import concourse.mybir as mybir
from concourse.bass_utils import run_bass_kernel_spmd
from concourse.masks import make_identity

F32 = mybir.dt.float32
BF16 = mybir.dt.bfloat16
F8 = mybir.dt.float8e4
AF = mybir.ActivationFunctionType
X = mybir.AxisListType.X
MULT = mybir.AluOpType.mult
ADD = mybir.AluOpType.add
SUB = mybir.AluOpType.subtract
MAXOP = mybir.AluOpType.max
BYPASS = mybir.AluOpType.bypass
DR = mybir.MatmulPerfMode.DoubleRow

B, CL, QL, H = 16, 512, 512, 768
N_CORES = 8
BPC = B // N_CORES      # batch elements per core
PC = CL // 128          # i-chunks (c rows)
QC = QL // 128          # j-chunks (q rows)
HC = H // 128           # h-chunks
LN_EPS = 1e-5
BF = ml_dtypes.bfloat16
E4M3 = ml_dtypes.float8_e4m3

# All activation funcs we use (Exp, Ln, Copy) live in the
# "natural_log_exp_and_others" table set. Blank out every other set so bass's
# table-load inserter emits exactly one load.
_ORIG_GAT = hw_specs.get_activation_tables


def _single_set_tables(arch):
    t = _ORIG_GAT(arch)
    return {
        name: (funcs if name == "natural_log_exp_and_others" else set())
        for name, funcs in t.items()
    }


bacc_mod.get_activation_tables = _single_set_tables


def build_kernel(use_mask: bool, trivial_ln: bool, reps: int = 1):
    nc = bacc.Bacc("TRN2", target_bir_lowering=False, debug=False)

    # ---- DRAM I/O (per-core shard shapes) ----
    d_cn = nc.dram_tensor("cn", [BPC, CL, H], BF16, kind="ExternalInput")
    d_cT = nc.dram_tensor("cT", [BPC, H, CL], BF16, kind="ExternalInput")
    d_cs8 = nc.dram_tensor("cs8", [BPC, H, CL], F8, kind="ExternalInput")
    d_cT8 = nc.dram_tensor("cT8", [BPC, H, CL], F8, kind="ExternalInput")
    d_qT8 = nc.dram_tensor("qT8", [BPC, H, QL], F8, kind="ExternalInput")
    d_qn8 = nc.dram_tensor("qn8", [BPC, QL, H], F8, kind="ExternalInput")
    d_wc8 = nc.dram_tensor("wc8", [128, HC, 128], F8, kind="ExternalInput")
    d_wq8 = nc.dram_tensor("wq8", [128, HC, 128], F8, kind="ExternalInput")
    d_waB = nc.dram_tensor("waB", [128, 2 * HC, H], BF16, kind="ExternalInput")
    d_waA = nc.dram_tensor("waA", [128, 2 * HC, H], F8, kind="ExternalInput")
    d_wab = nc.dram_tensor("wab", [1, H], BF16, kind="ExternalInput")
    if use_mask:
        d_mkT = nc.dram_tensor("mkT", [BPC, QL, CL], F32, kind="ExternalInput")
    if not trivial_ln:
        d_lng = nc.dram_tensor("lng", [H], F32, kind="ExternalInput")
        d_lnb = nc.dram_tensor("lnb", [H], F32, kind="ExternalInput")
    d_out = nc.dram_tensor("out", [BPC, CL, H], BF16, kind="ExternalOutput")

    with tile.TileContext(nc) as tc, ExitStack() as ctx:
        consts = ctx.enter_context(tc.tile_pool(name="consts", bufs=1))
        p_in = ctx.enter_context(tc.tile_pool(name="pin", bufs=2))
        p_work = ctx.enter_context(tc.tile_pool(name="work", bufs=2))
        p_small = ctx.enter_context(tc.tile_pool(name="small", bufs=2))
        p_y = ctx.enter_context(tc.tile_pool(name="ypool", bufs=2))
        # PSUM budget (8 banks of [128 x 2KB]):
        #   mm  [128,512]f32 x2 bufs = 2 banks; row (<=2KB) x2 = 2 banks
        #   A [128,768] x1 = 2 banks; B [128,768] x1 = 2 banks
        ps_mm = ctx.enter_context(tc.tile_pool(name="ps_mm", bufs=2, space="PSUM"))
        ps_row = ctx.enter_context(tc.tile_pool(name="ps_row", bufs=2, space="PSUM"))
        ps_a = ctx.enter_context(tc.tile_pool(name="ps_a", bufs=1, space="PSUM"))
        ps_b = ctx.enter_context(tc.tile_pool(name="ps_b", bufs=1, space="PSUM"))

        # ---- constants (once per core) ----
        waB = consts.tile([128, 2 * HC, H], BF16)      # [wa1 | wa4]
        nc.gpsimd.dma_start(waB[:], d_waB.ap()[:])
        waA = consts.tile([128, 2 * HC, H], F8)        # 64*[wa2 | wa3]
        nc.gpsimd.dma_start(waA[:], d_waA.ap()[:])
        wc8 = consts.tile([128, HC, 128], F8)          # 64*wc cols in M-col 0
        nc.gpsimd.dma_start(wc8[:], d_wc8.ap()[:])
        wq8 = consts.tile([128, HC, 128], F8)          # 64*wq cols in M-col 0
        nc.gpsimd.dma_start(wq8[:], d_wq8.ap()[:])
        wab_pad = consts.tile([128, H], BF16)
        nc.vector.memset(wab_pad[:], 0.0)
        nc.gpsimd.dma_start(wab_pad[0:1, :], d_wab.ap()[:])
        id_bf = consts.tile([128, 128], BF16)
        make_identity(nc, id_bf[:])
        ones_t = consts.tile([128, 128], BF16)         # row-0 ones (bias bcast)
        nc.vector.memset(ones_t[:], 0.0)
        nc.vector.memset(ones_t[0:1, :], 1.0)
        ones8 = consts.tile([128, 2, 128], F8)         # M-col 0 ones (rowsum)
        nc.vector.memset(ones8[:], 0.0)
        nc.vector.memset(ones8[:, :, 0:1], 1.0)
        ones_col = consts.tile([128, 1], BF16)         # all-ones column
        nc.vector.memset(ones_col[:], 1.0)
        c64i = consts.tile([1, 1], F32)                # 1/64  (unscale rows)
        nc.vector.memset(c64i[:], 1.0 / 64.0)
        c05 = consts.tile([1, 1], F32)                 # 0.5   (1/(fp8 scale S))
        nc.vector.memset(c05[:], 0.5)
        eps_t = consts.tile([128, 1], F32)
        nc.vector.memset(eps_t[:], LN_EPS)
        if not trivial_ln:
            g_bc = consts.tile([128, H], F32)
            nc.gpsimd.dma_start(
                g_bc[:], bass.AP(tensor=d_lng, offset=0, ap=[[0, 128], [1, H]])
            )
            b_bc = consts.tile([128, H], F32)
            nc.gpsimd.dma_start(
                b_bc[:], bass.AP(tensor=d_lnb, offset=0, ap=[[0, 128], [1, H]])
            )

        st = [dict() for _ in range(BPC)]

        def s_load(b):
            s = st[b]
            s["qT8"] = p_in.tile([128, HC, QL], F8, tag="qT8", name=f"qT8_{b}")
            nc.sync.dma_start(s["qT8"][:], d_qT8.ap()[b].rearrange("(o p) j -> p o j", p=128))
            s["cs8"] = p_in.tile([128, HC, CL], F8, tag="cs8", name=f"cs8_{b}")
            nc.sync.dma_start(s["cs8"][:], d_cs8.ap()[b].rearrange("(o p) i -> p o i", p=128))
            s["cT8"] = p_in.tile([128, HC, CL], F8, tag="cT8", name=f"cT8_{b}")
            nc.scalar.dma_start(s["cT8"][:], d_cT8.ap()[b].rearrange("(o p) i -> p o i", p=128))
            s["cTb"] = p_in.tile([128, HC, CL], BF16, tag="cTb", name=f"cTb_{b}")
            nc.scalar.dma_start(s["cTb"][:], d_cT.ap()[b].rearrange("(o p) i -> p o i", p=128))
            s["qn8"] = p_in.tile([128, QC, H], F8, tag="qn8", name=f"qn8_{b}")
            nc.sync.dma_start(s["qn8"][:], d_qn8.ap()[b].rearrange("(o p) h -> p o h", p=128))
            s["cn"] = p_in.tile([128, PC, H], BF16, tag="cn", name=f"cn_{b}")
            nc.scalar.dma_start(s["cn"][:], d_cn.ap()[b].rearrange("(o p) h -> p o h", p=128))
            if use_mask:
                s["mkT"] = p_in.tile([128, QC, CL], F32, tag="mkT", name=f"mkT_{b}")
                nc.sync.dma_start(
                    s["mkT"][:], d_mkT.ap()[b].rearrange("(o p) i -> p o i", p=128)
                )

        def s_front(b):
            # s_q columns: 64*s_q row (padded fp8 DR) -> cols * 1/64
            s = st[b]
            sq_ps = ps_mm.tile([128, QL], F32, tag="mm", name=f"sq_ps_{b}")
            for k in range(0, HC, 2):
                nc.tensor.matmul(
                    sq_ps[:], lhsT=wq8[:, k : k + 2], rhs=s["qT8"][:, k : k + 2],
                    start=(k == 0), stop=(k == HC - 2), perf_mode=DR,
                )
            sq_sb = p_small.tile([1, QL], F32, tag="sq_sb", name=f"sq_sb_{b}")
            nc.scalar.copy(sq_sb[:], sq_ps[0:1, :])
            sqc_ps = ps_row.tile([128, QC], F32, tag="row", name=f"sqc_ps_{b}")
            for jc in range(QC):
                nc.tensor.matmul(
                    sqc_ps[:, jc : jc + 1],
                    lhsT=sq_sb[0:1, jc * 128 : (jc + 1) * 128],
                    rhs=c64i[:], start=True, stop=True,
                )
            s["sqc"] = p_small.tile([128, QC], F32, tag="sqc", name=f"sqc_{b}")
            nc.scalar.copy(s["sqc"][:], sqc_ps[:])

        def s_logits(b):
            # s0T per j-chunk (fp8 DR) -> E^T = exp(s0T/64 + s_q) fp8
            s = st[b]
            s["ET8"] = p_work.tile([128, QC, CL], F8, tag="ET8", name=f"ET8_{b}")
            for jc in range(QC):
                s0 = ps_mm.tile([128, CL], F32, tag="mm", name=f"s0_{b}_{jc}")
                for k in range(0, HC, 2):
                    nc.tensor.matmul(
                        s0[:],
                        lhsT=s["qT8"][:, k : k + 2, jc * 128 : (jc + 1) * 128],
                        rhs=s["cs8"][:, k : k + 2],
                        start=(k == 0), stop=(k == HC - 2), perf_mode=DR,
                    )
                if use_mask:
                    nc.vector.tensor_add(s0[:], s0[:], s["mkT"][:, jc])
                nc.scalar.activation(
                    out=s["ET8"][:, jc], in_=s0[:], func=AF.Exp,
                    bias=s["sqc"][:, jc : jc + 1], scale=1.0 / 64.0,
                )

        def s_bpath(b):
            s = st[b]
            ET8 = s["ET8"]
            # rowsum rs[i] (padded fp8 DR ones) -> rr = 1/rs -> cols * 0.5
            rs_ps = ps_mm.tile([128, CL], F32, tag="mm", name=f"rs_ps_{b}")
            for k in range(0, QC, 2):
                nc.tensor.matmul(
                    rs_ps[:], lhsT=ones8[:], rhs=ET8[:, k : k + 2],
                    start=(k == 0), stop=(k == QC - 2), perf_mode=DR,
                )
            rr_row = p_small.tile([1, CL], F32, tag="rr_row", name=f"rr_row_{b}")
            nc.vector.reciprocal(rr_row[:], rs_ps[0:1, :])
            rrc_ps = ps_row.tile([128, PC], F32, tag="row", name=f"rrc_ps_{b}")
            for ic in range(PC):
                nc.tensor.matmul(
                    rrc_ps[:, ic : ic + 1],
                    lhsT=rr_row[0:1, ic * 128 : (ic + 1) * 128],
                    rhs=c05[:], start=True, stop=True,
                )
            s["rrS"] = p_small.tile([128, PC], F32, tag="rrS", name=f"rrS_{b}")
            nc.scalar.copy(s["rrS"][:], rrc_ps[:])

            # column max over j (for b): max_j E^T via TT-max + transposes
            mx1 = p_small.tile([128, CL], BF16, tag="mx1", name=f"mx1_{b}")
            mx2 = p_small.tile([128, CL], BF16, tag="mx2", name=f"mx2_{b}")
            nc.vector.tensor_tensor(mx1[:], ET8[:, 0], ET8[:, 1], op=MAXOP)
            nc.vector.tensor_tensor(mx2[:], ET8[:, 2], ET8[:, 3], op=MAXOP)
            nc.vector.tensor_tensor(mx1[:], mx1[:], mx2[:], op=MAXOP)
            mxE = p_small.tile([128, PC], F32, tag="mxE", name=f"mxE_{b}")
            for ic in range(PC):
                mt_ps = ps_row.tile([128, 128], F32, tag="row", name=f"mt_{b}_{ic}")
                nc.tensor.matmul(
                    mt_ps[:], lhsT=mx1[:, ic * 128 : (ic + 1) * 128],
                    rhs=id_bf[:], start=True, stop=True,
                )
                nc.vector.tensor_reduce(
                    out=mxE[:, ic : ic + 1], in_=mt_ps[:], axis=X, op=MAXOP
                )

            # b (unnormalized): eb = exp(s_c) * maxE; rb = 1/sum(eb)
            sc_ps = ps_mm.tile([128, CL], F32, tag="mm", name=f"sc_ps_{b}")
            for k in range(0, HC, 2):
                nc.tensor.matmul(
                    sc_ps[:], lhsT=wc8[:, k : k + 2], rhs=s["cT8"][:, k : k + 2],
                    start=(k == 0), stop=(k == HC - 2), perf_mode=DR,
                )
            sc_sb = p_small.tile([1, CL], F32, tag="sc_sb", name=f"sc_sb_{b}")
            nc.scalar.copy(sc_sb[:], sc_ps[0:1, :])
            scc_ps = ps_row.tile([128, PC], F32, tag="row", name=f"scc_ps_{b}")
            for ic in range(PC):
                nc.tensor.matmul(
                    scc_ps[:, ic : ic + 1],
                    lhsT=sc_sb[0:1, ic * 128 : (ic + 1) * 128],
                    rhs=c64i[:], start=True, stop=True,
                )
            esc = p_small.tile([128, PC], F32, tag="esc", name=f"esc_{b}")
            nc.scalar.activation(out=esc[:], in_=scc_ps[:], func=AF.Exp)
            s["eb"] = p_small.tile([128, PC], BF16, tag="eb", name=f"eb_{b}")
            nc.vector.tensor_tensor(s["eb"][:], esc[:], mxE[:], op=MULT)
            ebs_ps = ps_row.tile([1, PC], F32, tag="row", name=f"ebs_{b}")
            nc.tensor.matmul(ebs_ps[:], lhsT=ones_col[:], rhs=s["eb"][:],
                             start=True, stop=True)
            Sb = p_small.tile([1, 1], F32, tag="Sb", name=f"Sb_{b}")
            nc.vector.tensor_reduce(out=Sb[:], in_=ebs_ps[:], axis=X, op=ADD)
            s["rb"] = p_small.tile([1, 1], F32, tag="rb", name=f"rb_{b}")
            nc.vector.reciprocal(s["rb"][:], Sb[:])

        def s_c2q(b):
            # c2qT (unnorm, fp8 DR) -> c2qT8 = /32; xc8 = cT8 * c2qT8 (Pool)
            s = st[b]
            s["c2qT8"] = p_work.tile([128, HC, CL], F8, tag="c2qT8", name=f"c2qT8_{b}")
            s["xc8"] = p_work.tile([128, HC, CL], F8, tag="xc8", name=f"xc8_{b}")
            for hc in range(HC):
                cq_ps = ps_mm.tile([128, CL], F32, tag="mm", name=f"cq_{b}_{hc}")
                for k in range(0, QC, 2):
                    nc.tensor.matmul(
                        cq_ps[:],
                        lhsT=s["qn8"][:, k : k + 2, hc * 128 : (hc + 1) * 128],
                        rhs=s["ET8"][:, k : k + 2],
                        start=(k == 0), stop=(k == QC - 2), perf_mode=DR,
                    )
                nc.scalar.activation(
                    out=s["c2qT8"][:, hc], in_=cq_ps[:], func=AF.Copy,
                    scale=1.0 / 32.0,
                )
                nc.gpsimd.tensor_tensor(
                    s["xc8"][:, hc], s["cT8"][:, hc], s["c2qT8"][:, hc], op=MULT
                )

        def s_q2c(b):
            # q2c row = eb @ c (bf16); cols scaled by rb; merged = wa1+q2c.*wa4
            s = st[b]
            q2c_sb = p_small.tile([1, H], F32, tag="q2c_sb", name=f"q2c_sb_{b}")
            for n0, nw in ((0, 512), (512, 256)):
                qp = ps_row.tile([1, nw], F32, tag="row", name=f"qp_{b}_{n0}")
                for ic in range(PC):
                    nc.tensor.matmul(
                        qp[:], lhsT=s["eb"][:, ic : ic + 1],
                        rhs=s["cn"][:, ic, n0 : n0 + nw],
                        start=(ic == 0), stop=(ic == PC - 1),
                    )
                nc.scalar.copy(q2c_sb[0:1, n0 : n0 + nw], qp[:])
            qcc_ps = ps_row.tile([128, HC], F32, tag="row", name=f"qcc_ps_{b}")
            for hc in range(HC):
                nc.tensor.matmul(
                    qcc_ps[:, hc : hc + 1],
                    lhsT=q2c_sb[0:1, hc * 128 : (hc + 1) * 128],
                    rhs=s["rb"][:], start=True, stop=True,
                )
            q2c_c = p_small.tile([128, HC], F32, tag="q2c_c", name=f"q2c_c_{b}")
            nc.scalar.copy(q2c_c[:], qcc_ps[:])
            s["merged"] = p_work.tile([128, HC, H], BF16, tag="merged", name=f"merged_{b}")
            for hc in range(HC):
                nc.vector.scalar_tensor_tensor(
                    out=s["merged"][:, hc], in0=waB[:, HC + hc],
                    scalar=q2c_c[:, hc : hc + 1], in1=waB[:, hc],
                    op0=MULT, op1=ADD,
                )

        def s_big(b):
            # final matmul: B group (bf16 c@merged + bias), A group (fp8 DR)
            s = st[b]
            s["yt"] = p_y.tile([128, PC, H], BF16, tag="y", name=f"yt_{b}")
            s["ysum"] = p_small.tile([128, PC], F32, tag="ysum", name=f"ysum_{b}")
            s["ysq"] = p_small.tile([128, PC], F32, tag="ysq", name=f"ysq_{b}")
            vt = p_y.tile([128, H], BF16, tag="vt", name=f"vt_{b}")
            av = p_y.tile([128, H], BF16, tag="av", name=f"av_{b}")
            for ic in range(PC):
                pb = ps_b.tile([128, H], F32, tag="gb", name=f"gb_{b}_{ic}")
                pa = ps_a.tile([128, H], F32, tag="ga", name=f"ga_{b}_{ic}")
                for n0, nw in ((0, 512), (512, 256)):
                    k2 = 0
                    for comp, w0 in ((s["c2qT8"], 0), (s["xc8"], HC)):
                        for k in range(0, HC, 2):
                            nc.tensor.matmul(
                                pa[:, n0 : n0 + nw],
                                lhsT=comp[:, k : k + 2, ic * 128 : (ic + 1) * 128],
                                rhs=waA[:, w0 + k : w0 + k + 2, n0 : n0 + nw],
                                start=(k2 == 0), stop=(k2 == 2 * HC - 4),
                                perf_mode=DR, skip_group_check=True,
                            )
                            k2 += 2
                    for hc in range(HC):
                        nc.tensor.matmul(
                            pb[:, n0 : n0 + nw],
                            lhsT=s["cTb"][:, hc, ic * 128 : (ic + 1) * 128],
                            rhs=s["merged"][:, hc, n0 : n0 + nw],
                            start=(hc == 0), stop=False,
                            skip_group_check=True,
                        )
                    nc.tensor.matmul(
                        pb[:, n0 : n0 + nw], lhsT=ones_t[:],
                        rhs=wab_pad[:, n0 : n0 + nw],
                        start=False, stop=True, skip_group_check=True,
                    )
                # v = bigA * (rr/2) + bigB; y = relu(v) + c; accum y, y^2
                nc.scalar.activation(
                    out=av[:], in_=pa[:], func=AF.Copy,
                    scale=s["rrS"][:, ic : ic + 1],
                )
                nc.vector.scalar_tensor_tensor(
                    out=vt[:], in0=pb[:], scalar=0.0, in1=av[:],
                    op0=BYPASS, op1=ADD,
                )
                nc.vector.scalar_tensor_tensor(
                    out=s["yt"][:, ic], in0=vt[:], scalar=0.0, in1=s["cn"][:, ic],
                    op0=MAXOP, op1=ADD, accum_out=s["ysum"][:, ic : ic + 1],
                )
                nc.vector.scalar_tensor_tensor(
                    out=vt[:], in0=s["yt"][:, ic], scalar=1.0, in1=s["yt"][:, ic],
                    op0=BYPASS, op1=MULT, accum_out=s["ysq"][:, ic : ic + 1],
                )

        def s_ln(b):
            s = st[b]
            mu = p_small.tile([128, PC], F32, tag="mu", name=f"mu_{b}")
            nc.scalar.activation(out=mu[:], in_=s["ysum"][:], func=AF.Copy,
                                 scale=1.0 / H)
            var = p_small.tile([128, PC], F32, tag="var", name=f"var_{b}")
            nc.vector.scalar_tensor_tensor(
                out=var[:], in0=mu[:], scalar=1.0, in1=mu[:], op0=BYPASS, op1=MULT
            )
            nc.vector.scalar_tensor_tensor(
                out=var[:], in0=s["ysq"][:], scalar=1.0 / H, in1=var[:],
                op0=MULT, op1=SUB,
            )
            lnv = p_small.tile([128, PC], F32, tag="lnv", name=f"lnv_{b}")
            nc.scalar.activation(
                out=lnv[:], in_=var[:], func=AF.Ln, bias=eps_t[:], scale=1.0
            )
            rstd = p_small.tile([128, PC], F32, tag="rstd", name=f"rstd_{b}")
            nc.scalar.activation(out=rstd[:], in_=lnv[:], func=AF.Exp, scale=-0.5)
            ot = p_y.tile([128, PC, H], BF16, tag="ot", name=f"ot_{b}")
            for ic in range(PC):
                nc.vector.tensor_scalar(
                    out=ot[:, ic], in0=s["yt"][:, ic],
                    scalar1=mu[:, ic : ic + 1], scalar2=rstd[:, ic : ic + 1],
                    op0=SUB, op1=MULT,
                )
                if not trivial_ln:
                    nc.vector.tensor_tensor(ot[:, ic], ot[:, ic], g_bc[:], op=MULT)
                    nc.vector.tensor_add(ot[:, ic], ot[:, ic], b_bc[:])
                nc.sync.dma_start(
                    d_out.ap()[b].rearrange("(o p) h -> p o h", p=128)[:, ic],
                    ot[:, ic],
                )

        rep_ctx = tc.For_i(0, reps, 1) if reps > 1 else None
        if rep_ctx is not None:
            rep_ctx.__enter__()

        s_load(0); s_load(1)
        s_front(0); s_front(1)
        s_logits(0); s_logits(1)
        s_bpath(0); s_c2q(0)
        s_bpath(1); s_c2q(1)
        s_q2c(0); s_q2c(1)
        s_big(0); s_ln(0)
        s_big(1); s_ln(1)

        if rep_ctx is not None:
            rep_ctx.__exit__(None, None, None)

    nc.compile()
    return nc


_KERNEL_CACHE = {}


def get_kernel(use_mask: bool, trivial_ln: bool):
    key = (use_mask, trivial_ln)
    if key not in _KERNEL_CACHE:
        _KERNEL_CACHE[key] = build_kernel(use_mask, trivial_ln)
    return _KERNEL_CACHE[key]


def prep_inputs(inputs):
    """Host-side layout prep: shard over batch, transpose/cast, weight reshape."""
    c = np.ascontiguousarray(np.asarray(inputs["inputs"], dtype=np.float32))
    q = np.ascontiguousarray(np.asarray(inputs["states"], dtype=np.float32))
    mask = np.asarray(inputs["attention_mask"], dtype=np.float32)[:, 0]
    use_mask = bool(np.any(mask))
    ln_g = np.asarray(inputs["ln_g"], dtype=np.float32)
    ln_b = np.asarray(inputs["ln_b"], dtype=np.float32)
    trivial_ln = bool(np.all(ln_g == 1.0) and np.all(ln_b == 0.0))

    cn = c.astype(BF)
    cT = np.ascontiguousarray(c.transpose(0, 2, 1))
    qT = np.ascontiguousarray(q.transpose(0, 2, 1))
    cTb = cT.astype(BF)
    cT8 = cT.astype(E4M3)
    qT8 = qT.astype(E4M3)
    qn8 = q.astype(E4M3)
    wcq = np.asarray(inputs["wcq_w"], np.float32)[0]
    cs8 = (cT * (64.0 * wcq)[None, :, None]).astype(E4M3)

    def colsP(w, scale):
        # [128, HC, 128] fp8 with the column vector in M-slot 0
        col = (np.asarray(w, np.float32)[0] * scale).reshape(HC, 128).T
        out = np.zeros((128, HC, 128), np.float32)
        out[:, :, 0] = col
        return out.astype(E4M3)

    wc8 = colsP(inputs["wc_w"], 64.0)
    wq8 = colsP(inputs["wq_w"], 64.0)
    waT = np.ascontiguousarray(
        np.asarray(inputs["wa_w"], np.float32).T.reshape(4 * HC, 128, H)
        .transpose(1, 0, 2)
    )
    waB = np.ascontiguousarray(
        np.concatenate([waT[:, :HC], waT[:, 3 * HC :]], axis=1)
    ).astype(BF)                          # [wa1 | wa4]
    waA = np.ascontiguousarray(64.0 * waT[:, HC : 3 * HC]).astype(E4M3)  # 64*[wa2|wa3]
    wab = np.asarray(inputs["wa_b"], np.float32).reshape(1, H).astype(BF)

    in_maps = []
    for k in range(N_CORES):
        sl = slice(k * BPC, (k + 1) * BPC)
        m = {
            "cn": cn[sl],
            "cT": cTb[sl],
            "cs8": cs8[sl],
            "cT8": cT8[sl],
            "qT8": qT8[sl],
            "qn8": qn8[sl],
            "wc8": wc8,
            "wq8": wq8,
            "waB": waB,
            "waA": waA,
            "wab": wab,
        }
        if use_mask:
            m["mkT"] = np.ascontiguousarray(64.0 * mask[sl].transpose(0, 2, 1))
        if not trivial_ln:
            m["lng"] = ln_g
            m["lnb"] = ln_b
        in_maps.append(m)
    return in_maps, use_mask, trivial_ln


def kernel(**inputs) -> np.ndarray:
    in_maps, use_mask, trivial_ln = prep_inputs(inputs)
    nc = get_kernel(use_mask, trivial_ln)
    res = run_bass_kernel_spmd(nc, in_maps, core_ids=list(range(N_CORES)))
    return np.concatenate(
        [np.asarray(res.results[k]["out"]).astype(np.float32) for k in range(N_CORES)],
        axis=0,
    )


# revision 18
# speedup vs baseline: 1.3365x; 1.1813x over previous
"""AttentionDAF Trainium2 kernel — data-parallel over batch across 8 NeuronCores.

Reference (per batch element, c=inputs (512,768), q=states (512,768)):
    s[i,j] = (c*wcq)@q^T + s_c[i] + s_q[j] + mask
    a = softmax_j(s);  c2q = a @ q
    b = softmax_i(max_j s);  q2c = b @ c (broadcast over rows)
    x = [c, c2q, c*c2q, c*q2c];  y = relu(x @ wa^T + wa_b) + c;  out = LN(y)

Structure:
  - Logits computed TRANSPOSED (s0T[j,i]) so s_q is a per-partition bias of
    the exp activation, which writes E^T = exp(s0T + s_q) directly as fp8.
  - Row normalization (1/rowsum) is deferred: unnormalized c2q_un/xc_un flow
    through the fp8 matmuls; the per-row scalar rr/2 applies at the PSUM
    combine (Activation copy with per-partition scale).
  - max_j s for the b-path = max_j E^T (exp monotonic): TT-max over j-chunks,
    identity-transpose blocks, free-axis max. b stays unnormalized; 1/sum(b)
    folds into the q2c column transpose.
  - fp8e4 DoubleRow matmuls (2 K-chunks/instr, ~2x bf16 PE throughput) for
    s0T, c2qT, row matmuls (M padded to 128), and the c2q/xc components of
    the final matmul. The precision-critical c-component (rhs = wa1 +
    q2c.*wa4, built via one STT per chunk) stays bf16. fp8 operands are
    pre-scaled into e4m3's normal range; scales cancel at the combine.
  - The two batch elements per core are emitted stage-interleaved so the
    in-order engine queues overlap batch 0's dependency stalls with batch 1
    work (and the For_i rep loop pipelines across iterations).
"""
import sys
from contextlib import ExitStack

if "/opt/trn_rl_repo" not in sys.path:
    sys.path.insert(0, "/opt/trn_rl_repo")

import numpy as np
import ml_dtypes

from concourse import bacc
import concourse.bacc as bacc_mod
import concourse.hw_specs as hw_specs
import concourse.bass as bass
import concourse.tile as tile
import concourse.mybir as mybir
from concourse.bass_utils import run_bass_kernel_spmd
from concourse.masks import make_identity

F32 = mybir.dt.float32
BF16 = mybir.dt.bfloat16
F8 = mybir.dt.float8e4
AF = mybir.ActivationFunctionType
X = mybir.AxisListType.X
MULT = mybir.AluOpType.mult
ADD = mybir.AluOpType.add
SUB = mybir.AluOpType.subtract
MAXOP = mybir.AluOpType.max
BYPASS = mybir.AluOpType.bypass
DR = mybir.MatmulPerfMode.DoubleRow

B, CL, QL, H = 16, 512, 512, 768
N_CORES = 8
BPC = B // N_CORES      # batch elements per core
PC = CL // 128          # i-chunks (c rows)
QC = QL // 128          # j-chunks (q rows)
HC = H // 128           # h-chunks
LN_EPS = 1e-5
BF = ml_dtypes.bfloat16
E4M3 = ml_dtypes.float8_e4m3

# All activation funcs we use (Exp, Ln, Copy) live in the
# "natural_log_exp_and_others" table set. Blank out every other set so bass's
# table-load inserter emits exactly one load.
_ORIG_GAT = hw_specs.get_activation_tables


def _single_set_tables(arch):
    t = _ORIG_GAT(arch)
    return {
        name: (funcs if name == "natural_log_exp_and_others" else set())
        for name, funcs in t.items()
    }


bacc_mod.get_activation_tables = _single_set_tables


def build_kernel(use_mask: bool, trivial_ln: bool, reps: int = 1):
    nc = bacc.Bacc("TRN2", target_bir_lowering=False, debug=False)

    # ---- DRAM I/O (per-core shard shapes) ----
    d_cn = nc.dram_tensor("cn", [BPC, CL, H], BF16, kind="ExternalInput")
    d_cT = nc.dram_tensor("cT", [BPC, H, CL], BF16, kind="ExternalInput")
    d_cs8 = nc.dram_tensor("cs8", [BPC, H, CL], F8, kind="ExternalInput")
    d_cT8 = nc.dram_tensor("cT8", [BPC, H, CL], F8, kind="ExternalInput")
    d_qT8 = nc.dram_tensor("qT8", [BPC, H, QL], F8, kind="ExternalInput")
    d_qn8 = nc.dram_tensor("qn8", [BPC, QL, H], F8, kind="ExternalInput")
    d_wc8 = nc.dram_tensor("wc8", [128, HC, 128], F8, kind="ExternalInput")
    d_wq8 = nc.dram_tensor("wq8", [128, HC, 128], F8, kind="ExternalInput")
    d_waB = nc.dram_tensor("waB", [128, 2 * HC, H], BF16, kind="ExternalInput")
    d_waA = nc.dram_tensor("waA", [128, 2 * HC, H], F8, kind="ExternalInput")
    d_wab = nc.dram_tensor("wab", [1, H], BF16, kind="ExternalInput")
    if use_mask:
        d_mkT = nc.dram_tensor("mkT", [BPC, QL, CL], F32, kind="ExternalInput")
    if not trivial_ln:
        d_lng = nc.dram_tensor("lng", [H], F32, kind="ExternalInput")
        d_lnb = nc.dram_tensor("lnb", [H], F32, kind="ExternalInput")
    d_out = nc.dram_tensor("out", [BPC, CL, H], BF16, kind="ExternalOutput")

    with tile.TileContext(nc) as tc, ExitStack() as ctx:
        consts = ctx.enter_context(tc.tile_pool(name="consts", bufs=1))
        p_in = ctx.enter_context(tc.tile_pool(name="pin", bufs=2))
        p_work = ctx.enter_context(tc.tile_pool(name="work", bufs=2))
        p_small = ctx.enter_context(tc.tile_pool(name="small", bufs=2))
        p_y = ctx.enter_context(tc.tile_pool(name="ypool", bufs=2))
        # PSUM budget (8 banks of [128 x 2KB]):
        #   mm  [128,512]f32 x2 bufs = 2 banks; row (<=2KB) x2 = 2 banks
        #   A [128,768] x1 = 2 banks; B [128,768] x1 = 2 banks
        ps_mm = ctx.enter_context(tc.tile_pool(name="ps_mm", bufs=2, space="PSUM"))
        ps_row = ctx.enter_context(tc.tile_pool(name="ps_row", bufs=2, space="PSUM"))
        ps_a = ctx.enter_context(tc.tile_pool(name="ps_a", bufs=1, space="PSUM"))
        ps_b = ctx.enter_context(tc.tile_pool(name="ps_b", bufs=1, space="PSUM"))

        # ---- constants (once per core; first-needed first, spread queues) ----
        wq8 = consts.tile([128, HC, 128], F8)          # 64*wq cols in M-col 0
        nc.gpsimd.dma_start(wq8[:], d_wq8.ap()[:])
        wc8 = consts.tile([128, HC, 128], F8)          # 64*wc cols in M-col 0
        nc.gpsimd.dma_start(wc8[:], d_wc8.ap()[:])
        waA = consts.tile([128, 2 * HC, H], F8)        # 64*[wa2 | wa3]
        nc.gpsimd.dma_start(waA[:], d_waA.ap()[:])
        waB = consts.tile([128, 2 * HC, H], BF16)      # [wa1 | wa4]
        nc.gpsimd.dma_start(waB[:], d_waB.ap()[:])
        wab_pad = consts.tile([128, H], BF16)
        nc.vector.memset(wab_pad[:], 0.0)
        nc.gpsimd.dma_start(wab_pad[0:1, :], d_wab.ap()[:])
        id_bf = consts.tile([128, 128], BF16)
        make_identity(nc, id_bf[:])
        ones_t = consts.tile([128, 128], BF16)         # row-0 ones (bias bcast)
        nc.vector.memset(ones_t[:], 0.0)
        nc.vector.memset(ones_t[0:1, :], 1.0)
        ones8 = consts.tile([128, 2, 128], F8)         # M-col 0 ones (rowsum)
        nc.vector.memset(ones8[:], 0.0)
        nc.vector.memset(ones8[:, :, 0:1], 1.0)
        ones_col = consts.tile([128, 1], BF16)         # all-ones column
        nc.vector.memset(ones_col[:], 1.0)
        c64i = consts.tile([1, 1], F32)                # 1/64  (unscale rows)
        nc.vector.memset(c64i[:], 1.0 / 64.0)
        c05 = consts.tile([1, 1], F32)                 # 0.5   (1/(fp8 scale S))
        nc.vector.memset(c05[:], 0.5)
        eps_t = consts.tile([128, 1], F32)
        nc.vector.memset(eps_t[:], LN_EPS)
        if not trivial_ln:
            g_bc = consts.tile([128, H], F32)
            nc.gpsimd.dma_start(
                g_bc[:], bass.AP(tensor=d_lng, offset=0, ap=[[0, 128], [1, H]])
            )
            b_bc = consts.tile([128, H], F32)
            nc.gpsimd.dma_start(
                b_bc[:], bass.AP(tensor=d_lnb, offset=0, ap=[[0, 128], [1, H]])
            )

        st = [dict() for _ in range(BPC)]

        def s_load(b):
            s = st[b]
            s["qT8"] = p_in.tile([128, HC, QL], F8, tag="qT8", name=f"qT8_{b}")
            nc.sync.dma_start(s["qT8"][:], d_qT8.ap()[b].rearrange("(o p) j -> p o j", p=128))
            s["cs8"] = p_in.tile([128, HC, CL], F8, tag="cs8", name=f"cs8_{b}")
            nc.sync.dma_start(s["cs8"][:], d_cs8.ap()[b].rearrange("(o p) i -> p o i", p=128))
            s["cT8"] = p_in.tile([128, HC, CL], F8, tag="cT8", name=f"cT8_{b}")
            nc.scalar.dma_start(s["cT8"][:], d_cT8.ap()[b].rearrange("(o p) i -> p o i", p=128))
            s["cTb"] = p_in.tile([128, HC, CL], BF16, tag="cTb", name=f"cTb_{b}")
            nc.scalar.dma_start(s["cTb"][:], d_cT.ap()[b].rearrange("(o p) i -> p o i", p=128))
            s["qn8"] = p_in.tile([128, QC, H], F8, tag="qn8", name=f"qn8_{b}")
            nc.sync.dma_start(s["qn8"][:], d_qn8.ap()[b].rearrange("(o p) h -> p o h", p=128))
            s["cn"] = p_in.tile([128, PC, H], BF16, tag="cn", name=f"cn_{b}")
            nc.scalar.dma_start(s["cn"][:], d_cn.ap()[b].rearrange("(o p) h -> p o h", p=128))
            if use_mask:
                s["mkT"] = p_in.tile([128, QC, CL], F32, tag="mkT", name=f"mkT_{b}")
                nc.sync.dma_start(
                    s["mkT"][:], d_mkT.ap()[b].rearrange("(o p) i -> p o i", p=128)
                )

        def s_front(b):
            # s_q columns: 64*s_q row (padded fp8 DR) -> cols * 1/64
            s = st[b]
            sq_ps = ps_mm.tile([128, QL], F32, tag="mm", name=f"sq_ps_{b}")
            for k in range(0, HC, 2):
                nc.tensor.matmul(
                    sq_ps[:], lhsT=wq8[:, k : k + 2], rhs=s["qT8"][:, k : k + 2],
                    start=(k == 0), stop=(k == HC - 2), perf_mode=DR,
                )
            sq_sb = p_small.tile([1, QL], F32, tag="sq_sb", name=f"sq_sb_{b}")
            nc.scalar.copy(sq_sb[:], sq_ps[0:1, :])
            sqc_ps = ps_row.tile([128, QC], F32, tag="row", name=f"sqc_ps_{b}")
            for jc in range(QC):
                nc.tensor.matmul(
                    sqc_ps[:, jc : jc + 1],
                    lhsT=sq_sb[0:1, jc * 128 : (jc + 1) * 128],
                    rhs=c64i[:], start=True, stop=True,
                )
            s["sqc"] = p_small.tile([128, QC], F32, tag="sqc", name=f"sqc_{b}")
            nc.scalar.copy(s["sqc"][:], sqc_ps[:])

        def s_logits(b):
            # s0T per j-chunk (fp8 DR) -> E^T = exp(s0T/64 + s_q) fp8
            s = st[b]
            s["ET8"] = p_work.tile([128, QC, CL], F8, tag="ET8", name=f"ET8_{b}")
            for jc in range(QC):
                s0 = ps_mm.tile([128, CL], F32, tag="mm", name=f"s0_{b}_{jc}")
                for k in range(0, HC, 2):
                    nc.tensor.matmul(
                        s0[:],
                        lhsT=s["qT8"][:, k : k + 2, jc * 128 : (jc + 1) * 128],
                        rhs=s["cs8"][:, k : k + 2],
                        start=(k == 0), stop=(k == HC - 2), perf_mode=DR,
                    )
                if use_mask:
                    nc.vector.tensor_add(s0[:], s0[:], s["mkT"][:, jc])
                nc.scalar.activation(
                    out=s["ET8"][:, jc], in_=s0[:], func=AF.Exp,
                    bias=s["sqc"][:, jc : jc + 1], scale=1.0 / 64.0,
                )

        def s_bpath(b):
            s = st[b]
            ET8 = s["ET8"]
            # rowsum rs[i] (padded fp8 DR ones) -> rr = 1/rs -> cols * 0.5
            rs_ps = ps_mm.tile([128, CL], F32, tag="mm", name=f"rs_ps_{b}")
            for k in range(0, QC, 2):
                nc.tensor.matmul(
                    rs_ps[:], lhsT=ones8[:], rhs=ET8[:, k : k + 2],
                    start=(k == 0), stop=(k == QC - 2), perf_mode=DR,
                )
            rr_row = p_small.tile([1, CL], F32, tag="rr_row", name=f"rr_row_{b}")
            nc.vector.reciprocal(rr_row[:], rs_ps[0:1, :])
            rrc_ps = ps_row.tile([128, PC], F32, tag="row", name=f"rrc_ps_{b}")
            for ic in range(PC):
                nc.tensor.matmul(
                    rrc_ps[:, ic : ic + 1],
                    lhsT=rr_row[0:1, ic * 128 : (ic + 1) * 128],
                    rhs=c05[:], start=True, stop=True,
                )
            s["rrS"] = p_small.tile([128, PC], F32, tag="rrS", name=f"rrS_{b}")
            nc.scalar.copy(s["rrS"][:], rrc_ps[:])

            # column max over j (for b): max_j E^T via TT-max + transposes
            mx1 = p_small.tile([128, CL], BF16, tag="mx1", name=f"mx1_{b}")
            mx2 = p_small.tile([128, CL], BF16, tag="mx2", name=f"mx2_{b}")
            nc.vector.tensor_tensor(mx1[:], ET8[:, 0], ET8[:, 1], op=MAXOP)
            nc.vector.tensor_tensor(mx2[:], ET8[:, 2], ET8[:, 3], op=MAXOP)
            nc.vector.tensor_tensor(mx1[:], mx1[:], mx2[:], op=MAXOP)
            mxE = p_small.tile([128, PC], F32, tag="mxE", name=f"mxE_{b}")
            for ic in range(PC):
                mt_ps = ps_row.tile([128, 128], F32, tag="row", name=f"mt_{b}_{ic}")
                nc.tensor.matmul(
                    mt_ps[:], lhsT=mx1[:, ic * 128 : (ic + 1) * 128],
                    rhs=id_bf[:], start=True, stop=True,
                )
                nc.vector.tensor_reduce(
                    out=mxE[:, ic : ic + 1], in_=mt_ps[:], axis=X, op=MAXOP
                )

            # b (unnormalized): eb = exp(s_c) * maxE; rb = 1/sum(eb)
            sc_ps = ps_mm.tile([128, CL], F32, tag="mm", name=f"sc_ps_{b}")
            for k in range(0, HC, 2):
                nc.tensor.matmul(
                    sc_ps[:], lhsT=wc8[:, k : k + 2], rhs=s["cT8"][:, k : k + 2],
                    start=(k == 0), stop=(k == HC - 2), perf_mode=DR,
                )
            sc_sb = p_small.tile([1, CL], F32, tag="sc_sb", name=f"sc_sb_{b}")
            nc.scalar.copy(sc_sb[:], sc_ps[0:1, :])
            scc_ps = ps_row.tile([128, PC], F32, tag="row", name=f"scc_ps_{b}")
            for ic in range(PC):
                nc.tensor.matmul(
                    scc_ps[:, ic : ic + 1],
                    lhsT=sc_sb[0:1, ic * 128 : (ic + 1) * 128],
                    rhs=c64i[:], start=True, stop=True,
                )
            esc = p_small.tile([128, PC], F32, tag="esc", name=f"esc_{b}")
            nc.scalar.activation(out=esc[:], in_=scc_ps[:], func=AF.Exp)
            s["eb"] = p_small.tile([128, PC], BF16, tag="eb", name=f"eb_{b}")
            nc.vector.tensor_tensor(s["eb"][:], esc[:], mxE[:], op=MULT)
            ebs_ps = ps_row.tile([1, PC], F32, tag="row", name=f"ebs_{b}")
            nc.tensor.matmul(ebs_ps[:], lhsT=ones_col[:], rhs=s["eb"][:],
                             start=True, stop=True)
            Sb = p_small.tile([1, 1], F32, tag="Sb", name=f"Sb_{b}")
            nc.vector.tensor_reduce(out=Sb[:], in_=ebs_ps[:], axis=X, op=ADD)
            s["rb"] = p_small.tile([1, 1], F32, tag="rb", name=f"rb_{b}")
            nc.vector.reciprocal(s["rb"][:], Sb[:])

        def s_c2q(b):
            # c2qT (unnorm, fp8 DR) -> c2qT8 = /32; xc8 = cT8 * c2qT8 (Pool)
            s = st[b]
            s["c2qT8"] = p_work.tile([128, HC, CL], F8, tag="c2qT8", name=f"c2qT8_{b}")
            s["xc8"] = p_work.tile([128, HC, CL], F8, tag="xc8", name=f"xc8_{b}")
            for hc in range(HC):
                cq_ps = ps_mm.tile([128, CL], F32, tag="mm", name=f"cq_{b}_{hc}")
                for k in range(0, QC, 2):
                    nc.tensor.matmul(
                        cq_ps[:],
                        lhsT=s["qn8"][:, k : k + 2, hc * 128 : (hc + 1) * 128],
                        rhs=s["ET8"][:, k : k + 2],
                        start=(k == 0), stop=(k == QC - 2), perf_mode=DR,
                    )
                nc.scalar.activation(
                    out=s["c2qT8"][:, hc], in_=cq_ps[:], func=AF.Copy,
                    scale=1.0 / 32.0,
                )
                nc.gpsimd.tensor_tensor(
                    s["xc8"][:, hc], s["cT8"][:, hc], s["c2qT8"][:, hc], op=MULT
                )

        def s_q2c(b):
            # q2c row = eb @ c (bf16); cols scaled by rb; merged = wa1+q2c.*wa4
            s = st[b]
            q2c_sb = p_small.tile([1, H], F32, tag="q2c_sb", name=f"q2c_sb_{b}")
            for n0, nw in ((0, 512), (512, 256)):
                qp = ps_row.tile([1, nw], F32, tag="row", name=f"qp_{b}_{n0}")
                for ic in range(PC):
                    nc.tensor.matmul(
                        qp[:], lhsT=s["eb"][:, ic : ic + 1],
                        rhs=s["cn"][:, ic, n0 : n0 + nw],
                        start=(ic == 0), stop=(ic == PC - 1),
                    )
                nc.scalar.copy(q2c_sb[0:1, n0 : n0 + nw], qp[:])
            qcc_ps = ps_row.tile([128, HC], F32, tag="row", name=f"qcc_ps_{b}")
            for hc in range(HC):
                nc.tensor.matmul(
                    qcc_ps[:, hc : hc + 1],
                    lhsT=q2c_sb[0:1, hc * 128 : (hc + 1) * 128],
                    rhs=s["rb"][:], start=True, stop=True,
                )
            q2c_c = p_small.tile([128, HC], F32, tag="q2c_c", name=f"q2c_c_{b}")
            nc.scalar.copy(q2c_c[:], qcc_ps[:])
            s["merged"] = p_work.tile([128, HC, H], BF16, tag="merged", name=f"merged_{b}")
            for hc in range(HC):
                nc.vector.scalar_tensor_tensor(
                    out=s["merged"][:, hc], in0=waB[:, HC + hc],
                    scalar=q2c_c[:, hc : hc + 1], in1=waB[:, hc],
                    op0=MULT, op1=ADD,
                )

        def s_big_init(b):
            s = st[b]
            s["yt"] = p_y.tile([128, PC, H], BF16, tag="y", name=f"yt_{b}")
            s["ysum"] = p_small.tile([128, PC], F32, tag="ysum", name=f"ysum_{b}")
            s["ysq"] = p_small.tile([128, PC], F32, tag="ysq", name=f"ysq_{b}")
            s["vt"] = p_y.tile([128, H], BF16, tag="vt", name=f"vt_{b}")
            s["av"] = p_y.tile([128, H], BF16, tag="av", name=f"av_{b}")

        def s_big_ic(b, ic):
            # final matmul: B group (bf16 c@merged + bias), A group (fp8 DR)
            s = st[b]
            vt, av = s["vt"], s["av"]
            if True:
                pb = ps_b.tile([128, H], F32, tag="gb", name=f"gb_{b}_{ic}")
                pa = ps_a.tile([128, H], F32, tag="ga", name=f"ga_{b}_{ic}")
                for n0, nw in ((0, 512), (512, 256)):
                    k2 = 0
                    for comp, w0 in ((s["c2qT8"], 0), (s["xc8"], HC)):
                        for k in range(0, HC, 2):
                            nc.tensor.matmul(
                                pa[:, n0 : n0 + nw],
                                lhsT=comp[:, k : k + 2, ic * 128 : (ic + 1) * 128],
                                rhs=waA[:, w0 + k : w0 + k + 2, n0 : n0 + nw],
                                start=(k2 == 0), stop=(k2 == 2 * HC - 4),
                                perf_mode=DR, skip_group_check=True,
                            )
                            k2 += 2
                    for hc in range(HC):
                        nc.tensor.matmul(
                            pb[:, n0 : n0 + nw],
                            lhsT=s["cTb"][:, hc, ic * 128 : (ic + 1) * 128],
                            rhs=s["merged"][:, hc, n0 : n0 + nw],
                            start=(hc == 0), stop=False,
                            skip_group_check=True,
                        )
                    nc.tensor.matmul(
                        pb[:, n0 : n0 + nw], lhsT=ones_t[:],
                        rhs=wab_pad[:, n0 : n0 + nw],
                        start=False, stop=True, skip_group_check=True,
                    )
                # v = bigA * (rr/2) + bigB; y = relu(v) + c; accum y, y^2
                nc.scalar.activation(
                    out=av[:], in_=pa[:], func=AF.Copy,
                    scale=s["rrS"][:, ic : ic + 1],
                )
                nc.vector.scalar_tensor_tensor(
                    out=vt[:], in0=pb[:], scalar=0.0, in1=av[:],
                    op0=BYPASS, op1=ADD,
                )
                nc.vector.scalar_tensor_tensor(
                    out=s["yt"][:, ic], in0=vt[:], scalar=0.0, in1=s["cn"][:, ic],
                    op0=MAXOP, op1=ADD, accum_out=s["ysum"][:, ic : ic + 1],
                )
                nc.vector.scalar_tensor_tensor(
                    out=vt[:], in0=s["yt"][:, ic], scalar=1.0, in1=s["yt"][:, ic],
                    op0=BYPASS, op1=MULT, accum_out=s["ysq"][:, ic : ic + 1],
                )

        def s_ln(b):
            s = st[b]
            mu = p_small.tile([128, PC], F32, tag="mu", name=f"mu_{b}")
            nc.scalar.activation(out=mu[:], in_=s["ysum"][:], func=AF.Copy,
                                 scale=1.0 / H)
            var = p_small.tile([128, PC], F32, tag="var", name=f"var_{b}")
            nc.vector.scalar_tensor_tensor(
                out=var[:], in0=mu[:], scalar=1.0, in1=mu[:], op0=BYPASS, op1=MULT
            )
            nc.vector.scalar_tensor_tensor(
                out=var[:], in0=s["ysq"][:], scalar=1.0 / H, in1=var[:],
                op0=MULT, op1=SUB,
            )
            lnv = p_small.tile([128, PC], F32, tag="lnv", name=f"lnv_{b}")
            nc.scalar.activation(
                out=lnv[:], in_=var[:], func=AF.Ln, bias=eps_t[:], scale=1.0
            )
            rstd = p_small.tile([128, PC], F32, tag="rstd", name=f"rstd_{b}")
            nc.scalar.activation(out=rstd[:], in_=lnv[:], func=AF.Exp, scale=-0.5)
            ot = p_y.tile([128, PC, H], BF16, tag="ot", name=f"ot_{b}")
            for ic in range(PC):
                nc.vector.tensor_scalar(
                    out=ot[:, ic], in0=s["yt"][:, ic],
                    scalar1=mu[:, ic : ic + 1], scalar2=rstd[:, ic : ic + 1],
                    op0=SUB, op1=MULT,
                )
                if not trivial_ln:
                    nc.vector.tensor_tensor(ot[:, ic], ot[:, ic], g_bc[:], op=MULT)
                    nc.vector.tensor_add(ot[:, ic], ot[:, ic], b_bc[:])
                nc.sync.dma_start(
                    d_out.ap()[b].rearrange("(o p) h -> p o h", p=128)[:, ic],
                    ot[:, ic],
                )

        rep_ctx = tc.For_i(0, reps, 1) if reps > 1 else None
        if rep_ctx is not None:
            rep_ctx.__enter__()

        s_load(0); s_load(1)
        s_front(0); s_front(1)
        s_logits(0); s_logits(1)
        s_bpath(0); s_c2q(0)
        s_bpath(1); s_c2q(1)
        s_q2c(0); s_q2c(1)
        s_big_init(0); s_big_init(1)
        for ic in range(PC):
            s_big_ic(0, ic)
            s_big_ic(1, ic)
        s_ln(0); s_ln(1)

        if rep_ctx is not None:
            rep_ctx.__exit__(None, None, None)

    nc.compile()
    return nc


_KERNEL_CACHE = {}


def get_kernel(use_mask: bool, trivial_ln: bool):
    key = (use_mask, trivial_ln)
    if key not in _KERNEL_CACHE:
        _KERNEL_CACHE[key] = build_kernel(use_mask, trivial_ln)
    return _KERNEL_CACHE[key]


def prep_inputs(inputs):
    """Host-side layout prep: shard over batch, transpose/cast, weight reshape."""
    c = np.ascontiguousarray(np.asarray(inputs["inputs"], dtype=np.float32))
    q = np.ascontiguousarray(np.asarray(inputs["states"], dtype=np.float32))
    mask = np.asarray(inputs["attention_mask"], dtype=np.float32)[:, 0]
    use_mask = bool(np.any(mask))
    ln_g = np.asarray(inputs["ln_g"], dtype=np.float32)
    ln_b = np.asarray(inputs["ln_b"], dtype=np.float32)
    trivial_ln = bool(np.all(ln_g == 1.0) and np.all(ln_b == 0.0))

    cn = c.astype(BF)
    cT = np.ascontiguousarray(c.transpose(0, 2, 1))
    qT = np.ascontiguousarray(q.transpose(0, 2, 1))
    cTb = cT.astype(BF)
    cT8 = cT.astype(E4M3)
    qT8 = qT.astype(E4M3)
    qn8 = q.astype(E4M3)
    wcq = np.asarray(inputs["wcq_w"], np.float32)[0]
    cs8 = (cT * (64.0 * wcq)[None, :, None]).astype(E4M3)

    def colsP(w, scale):
        # [128, HC, 128] fp8 with the column vector in M-slot 0
        col = (np.asarray(w, np.float32)[0] * scale).reshape(HC, 128).T
        out = np.zeros((128, HC, 128), np.float32)
        out[:, :, 0] = col
        return out.astype(E4M3)

    wc8 = colsP(inputs["wc_w"], 64.0)
    wq8 = colsP(inputs["wq_w"], 64.0)
    waT = np.ascontiguousarray(
        np.asarray(inputs["wa_w"], np.float32).T.reshape(4 * HC, 128, H)
        .transpose(1, 0, 2)
    )
    waB = np.ascontiguousarray(
        np.concatenate([waT[:, :HC], waT[:, 3 * HC :]], axis=1)
    ).astype(BF)                          # [wa1 | wa4]
    waA = np.ascontiguousarray(64.0 * waT[:, HC : 3 * HC]).astype(E4M3)  # 64*[wa2|wa3]
    wab = np.asarray(inputs["wa_b"], np.float32).reshape(1, H).astype(BF)

    in_maps = []
    for k in range(N_CORES):
        sl = slice(k * BPC, (k + 1) * BPC)
        m = {
            "cn": cn[sl],
            "cT": cTb[sl],
            "cs8": cs8[sl],
            "cT8": cT8[sl],
            "qT8": qT8[sl],
            "qn8": qn8[sl],
            "wc8": wc8,
            "wq8": wq8,
            "waB": waB,
            "waA": waA,
            "wab": wab,
        }
        if use_mask:
            m["mkT"] = np.ascontiguousarray(64.0 * mask[sl].transpose(0, 2, 1))
        if not trivial_ln:
            m["lng"] = ln_g
            m["lnb"] = ln_b
        in_maps.append(m)
    return in_maps, use_mask, trivial_ln


def kernel(**inputs) -> np.ndarray:
    in_maps, use_mask, trivial_ln = prep_inputs(inputs)
    nc = get_kernel(use_mask, trivial_ln)
    res = run_bass_kernel_spmd(nc, in_maps, core_ids=list(range(N_CORES)))
    return np.concatenate(
        [np.asarray(res.results[k]["out"]).astype(np.float32) for k in range(N_CORES)],
        axis=0,
    )
